# revision 5
# baseline (speedup 1.0000x reference)
"""Self-contained Trainium2 Bass kernel for nn_CausalSelfAttention_59528246722829 (v4).

Sharding: 8 cores = 2 batches x 4 head-groups (4 heads / 256 channels each).
Each core computes Q/K/V projections for its head group, causal attention,
and a partial output projection.  The 4 partials per batch are summed with an
on-device ReduceScatter; each core returns a disjoint 256-row slice of the
batch output (host concatenates and adds the output bias).

v4 changes vs v3 (dispatch-overhead driven):
  - Measured per-dispatch cost scales with argument count (~30 us/arg) and
    argument bytes (~11-16 us/MiB); device compute hides under dispatch.
  - v3 packed all inputs into one blob (13 args -> 2).  v4 additionally
    dedupes the batch-shared operands (x^T, rope table, mask, identity):
    each core ships only a quarter of them and three AllGathers over the
    4-core batch group rebuild the full copies in DRAM scratch.
  - The output projection partial-sum moves on device: four ReduceScatters
    (one per 512-query chunk) replace the host-side sum, shrinking the
    output from 4 MiB to 1 MiB per core.
  - Per-core bytes: 10.6 MiB -> 4.2 MiB (blob 3.2 in + outT 1.0 out).
  - All collectives issue on the gpsimd queue ONLY (out-proj result DMAs
    moved to the sync queue) so a collective stalled on producer writes
    can never sit ahead of its own producers in the same queue.

v2 design (cost-model driven, unchanged):
  - S^T matmuls and exp trimmed to the causal triangle; [128,128] tri mask.
  - AV in natural [query, d] layout, 65-wide V tile with ones-column giving
    the softmax denominator; reciprocal + broadcast mult on PSUM evacuation.
  - y^T recovered with PE transposes; qc-outer / head-inner loop order;
    proj/outproj units woven into attention as PE filler at exp-wait stalls.

Device compute dtype: bf16 matmul operands, fp32 PSUM accumulation.
"""

import numpy as np
import ml_dtypes

BF = ml_dtypes.bfloat16
B, T, E, H, DH = 2, 2048, 1024, 16, 64
P, NE, CL, NCT = 128, 8, 256, 2
ROPE_BASE = 10000.0
N_CORES = 8
CORES = [(b, g) for b in range(B) for g in range(4)]  # (batch, head-group)
GROUPS = [[0, 1, 2, 3], [4, 5, 6, 7]]

# ---- gathered (batch-shared) regions, per-rank quarter sizes in elems ----
SH_R0 = E * 512 // 4        # x^T[:, 0:512] quarter     = 131072
SH_R1 = E * 1536 // 4       # x^T[:, 512:2048] quarter  = 393216
SH_R2 = (P * T + 2 * P * P) // 4  # rope|mask|ident quarter = 81920... computed below
SZ_ROPE, SZ_MASK = P * T, P * P
SH_R2 = (SZ_ROPE + 2 * SZ_MASK) // 4
OFF_R0, OFF_R1, OFF_R2 = 0, SH_R0, SH_R0 + SH_R1
OFF_W = OFF_R2 + SH_R2
# ---- per-core (head-group) region ----
SZ_WQ, SZ_WV, SZ_WO = E * CL, E * 260, CL * E
SZ_BQ, SZ_BV = P * NCT, P * 4 * 65
OFF_WQ = OFF_W
OFF_WK = OFF_WQ + SZ_WQ
OFF_WV = OFF_WK + SZ_WQ
OFF_WO = OFF_WV + SZ_WV
OFF_BQ = OFF_WO + SZ_WO
OFF_BK = OFF_BQ + SZ_BQ
OFF_BV = OFF_BK + SZ_BQ
SZ_BLOB = OFF_BV + SZ_BV


def _rope_ct():
    """C^T[p, t] = cos(theta) + sin(theta), theta = t * base^(-2*(p%32)/64).

    The reference's buggy rope (d_param = n_embd slices the heads axis)
    degenerates to an elementwise multiply of Q and K by this factor.
    """
    i = (np.arange(P) % 32).astype(np.float64)
    t = np.arange(T, dtype=np.float64)
    th = t[None, :] * (ROPE_BASE ** (-2.0 * i[:, None] / DH))
    return (np.cos(th) + np.sin(th)).astype(np.float32)


def _mask_tri():
    """mask[p, q] = 1 if q >= p else 0 -- the 128x128 diagonal block mask."""
    p = np.arange(P)[:, None]
    q = np.arange(P)[None, :]
    return (q >= p).astype(np.float32)


def build_nc(debug=False):
    import concourse.bass as bass
    import concourse.tile as tile
    from concourse import mybir, bacc
    from contextlib import ExitStack

    f32, bf16 = mybir.dt.float32, mybir.dt.bfloat16
    Exp = mybir.ActivationFunctionType.Exp

    nc = bacc.Bacc("TRN2")
    if debug:
        QTD = nc.declare_dram_parameter("QTD", [P, NCT, T], bf16, isOutput=True)
        KTD = nc.declare_dram_parameter("KTD", [P, NCT, T], bf16, isOutput=True)
        VsD = nc.declare_dram_parameter("VsD", [P, 16, 4, 65], bf16, isOutput=True)
        ysbD = nc.declare_dram_parameter("ysbD", [P, NCT, T], bf16, isOutput=True)
        denD = nc.declare_dram_parameter("denD", [4, P, 4, 4], f32, isOutput=True)
        yspD = nc.declare_dram_parameter("yspD", [4, P, NCT, 4, P], bf16, isOutput=True)
    blob = nc.declare_dram_parameter("blob", [SZ_BLOB], bf16, isOutput=False)
    outT = nc.declare_dram_parameter("outT", [CL, T], bf16, isOutput=True)

    def bsl(off, sz):
        return blob[off : off + sz]

    with ExitStack() as ctx:
        tc = ctx.enter_context(tile.TileContext(nc))
        cst = ctx.enter_context(tc.tile_pool(name="cst", bufs=1))
        dram = ctx.enter_context(tc.tile_pool(name="dram", bufs=1, space="DRAM"))

        # ---- Phase -1: AllGather the batch-shared operands ----
        # Gathered layouts (rank r contributes rows 256r..256r+255):
        #   xg0 [E, 512]   x^T first t-chunk     xg1 [E, 1536]  rest
        #   cg  [SZ_ROPE + 2*SZ_MASK] flat rope|mask|ident
        xg0 = dram.tile([4 * SH_R0], bf16, tag="xg0")
        xg1 = dram.tile([4 * SH_R1], bf16, tag="xg1")
        cg = dram.tile([4 * SH_R2], bf16, tag="cg")
        # collectives cannot read IO tensors: bounce the shard regions
        # into DRAM scratch first (HBM->HBM, ~1 GB total/s budget is fine)
        shb = dram.tile([OFF_W], bf16, tag="shb")
        nc.gpsimd.dma_start(shb[OFF_R0:OFF_R1], bsl(OFF_R0, SH_R0))
        nc.gpsimd.collective_compute(
            "AllGather", mybir.AluOpType.bypass, replica_groups=GROUPS,
            ins=[shb[OFF_R0:OFF_R1].opt()], outs=[xg0[:].opt()],
        )
        nc.gpsimd.dma_start(shb[OFF_R2:OFF_W], bsl(OFF_R2, SH_R2))
        nc.gpsimd.collective_compute(
            "AllGather", mybir.AluOpType.bypass, replica_groups=GROUPS,
            ins=[shb[OFF_R2:OFF_W].opt()], outs=[cg[:].opt()],
        )
        nc.gpsimd.dma_start(shb[OFF_R1:OFF_R2], bsl(OFF_R1, SH_R1))
        nc.gpsimd.collective_compute(
            "AllGather", mybir.AluOpType.bypass, replica_groups=GROUPS,
            ins=[shb[OFF_R1:OFF_R2].opt()], outs=[xg1[:].opt()],
        )

        # ---- Phase 0: stream inputs across the DGE queues ----
        # SP: wq + x chunks.  ACT: wk + x chunks.  Pool(gpsimd): collectives
        # first, then consts + wv + wo (all gated on the AGs anyway).
        xT = cst.tile([P, NE, T], bf16)
        x0src = xg0[:].rearrange("(et p t) -> p et t", p=P, t=512)
        x1src = xg1[:].rearrange("(et p t) -> p et t", p=P, t=1536)
        wT = {}
        wT["q"] = cst.tile([P, NE, CL], bf16, name="wTq", tag="wTq")
        nc.sync.dma_start(
            wT["q"][:], bsl(OFF_WQ, SZ_WQ).rearrange("(et p c) -> p et c", p=P, c=CL)
        )
        wT["k"] = cst.tile([P, NE, CL], bf16, name="wTk", tag="wTk")
        nc.scalar.dma_start(
            wT["k"][:], bsl(OFF_WK, SZ_WQ).rearrange("(et p c) -> p et c", p=P, c=CL)
        )
        # x tq0 split across both queues so the first projection starts ASAP
        nc.sync.dma_start(xT[:, 0:4, 0:512], x0src[:, 0:4, :])
        nc.scalar.dma_start(xT[:, 4:8, 0:512], x0src[:, 4:8, :])
        nc.sync.dma_start(xT[:, :, 512:1024], x1src[:, :, 0:512])
        nc.scalar.dma_start(xT[:, :, 1024:1536], x1src[:, :, 512:1024])
        nc.sync.dma_start(xT[:, :, 1536:2048], x1src[:, :, 1024:1536])
        ropeS = cst.tile([P, T], bf16)
        nc.gpsimd.dma_start(
            ropeS[:], cg[0:SZ_ROPE].rearrange("(p t) -> p t", p=P)
        )
        maskS = cst.tile([P, P], bf16)
        nc.gpsimd.dma_start(
            maskS[:], cg[SZ_ROPE : SZ_ROPE + SZ_MASK].rearrange("(p q) -> p q", p=P)
        )
        identS = cst.tile([P, P], bf16)
        nc.gpsimd.dma_start(
            identS[:],
            cg[SZ_ROPE + SZ_MASK : SZ_ROPE + 2 * SZ_MASK].rearrange(
                "(p q) -> p q", p=P
            ),
        )
        bqSb = cst.tile([P, NCT], bf16)
        nc.gpsimd.dma_start(bqSb[:], bsl(OFF_BQ, SZ_BQ).rearrange("(p n) -> p n", p=P))
        bkSb = cst.tile([P, NCT], bf16)
        nc.gpsimd.dma_start(bkSb[:], bsl(OFF_BK, SZ_BQ).rearrange("(p n) -> p n", p=P))
        bvSb = cst.tile([P, 4, 65], bf16)
        nc.gpsimd.dma_start(
            bvSb[:], bsl(OFF_BV, SZ_BV).rearrange("(p h d) -> p h d", p=P, h=4)
        )
        wvP = cst.tile([P, NE, 4, 65], bf16)
        nc.gpsimd.dma_start(
            wvP[:].rearrange("p et h d -> p et (h d)"),
            bsl(OFF_WV, SZ_WV).rearrange("(et p c) -> p et c", p=P, c=260),
        )
        woT = cst.tile([P, NCT, E], bf16)
        nc.gpsimd.dma_start(
            woT[:], bsl(OFF_WO, SZ_WO).rearrange("(ct p f) -> p ct f", p=P, f=E)
        )
        # up-convert biases to fp32 once (broadcast adds below want fp32)
        bqS = cst.tile([P, NCT], f32)
        nc.vector.tensor_copy(out=bqS[:], in_=bqSb[:])
        bkS = cst.tile([P, NCT], f32)
        nc.vector.tensor_copy(out=bkS[:], in_=bkSb[:])
        bvS = cst.tile([P, 4, 65], f32)
        nc.vector.tensor_copy(out=bvS[:], in_=bvSb[:])

        QT = cst.tile([P, NCT, T], bf16)
        KT = cst.tile([P, NCT, T], bf16)
        Vs = cst.tile([P, 16, 4, 65], bf16)
        ysb = cst.tile([P, NCT, T], bf16)
        # per-chunk out-proj partials (DRAM) and ReduceScatter results
        ypD = [
            dram.tile([E, 512], bf16, name=f"ypD{qc}", tag=f"ypD{qc}")
            for qc in range(4)
        ]
        ysD = [
            dram.tile([CL, 512], bf16, name=f"ysD{qc}", tag=f"ysD{qc}")
            for qc in range(4)
        ]

        # ---- Fused phases, t-chunk major (see module docstring) ----
        with (
            tc.tile_pool(name="pgen", bufs=2, space="PSUM") as pgen,
            tc.tile_pool(name="psS", bufs=2, space="PSUM") as psS,
            tc.tile_pool(name="psy", bufs=1, space="PSUM") as psy,
            tc.tile_pool(name="pst", bufs=1, space="PSUM") as pst,
            tc.tile_pool(name="stg1", bufs=4) as stg1,
            tc.tile_pool(name="ptp", bufs=25) as ptp,
            tc.tile_pool(name="ysp", bufs=2) as yspp,
            tc.tile_pool(name="rcp", bufs=4) as rcp,
            tc.tile_pool(name="ostg", bufs=4) as ostg,
        ):
            def proj_units(tq):
                units = []
                for nm, dst, bS in (("q", QT, bqS), ("k", KT, bkS)):
                    for ct in range(NCT):
                        def qk_unit(nm=nm, dst=dst, bS=bS, ct=ct):
                            ps = pgen.tile([P, 512], f32, tag="gen")
                            for et in range(NE):
                                nc.tensor.matmul(
                                    ps[:],
                                    wT[nm][:, et, ct * P : (ct + 1) * P],
                                    xT[:, et, tq * 512 : (tq + 1) * 512],
                                    start=(et == 0),
                                    stop=(et == NE - 1),
                                )
                            tmp = stg1.tile([P, 512], bf16)
                            nc.vector.tensor_add(
                                out=tmp[:],
                                in0=ps[:],
                                in1=bS[:, ct : ct + 1].to_broadcast((P, 512)),
                            )
                            nc.vector.tensor_mul(
                                out=dst[:, ct, tq * 512 : (tq + 1) * 512],
                                in0=tmp[:],
                                in1=ropeS[:, tq * 512 : (tq + 1) * 512],
                            )
                        units.append(qk_unit)
                for tt in range(4 * tq, 4 * tq + 4):
                    def v_unit(tt=tt):
                        ps = pgen.tile([P, 512], f32, tag="gen")
                        for et in range(NE):
                            nc.tensor.matmul(
                                ps[:, 0:260],
                                xT[:, et, tt * P : (tt + 1) * P],
                                wvP[:, et].rearrange("p h d -> p (h d)"),
                                start=(et == 0),
                                stop=(et == NE - 1),
                            )
                        nc.vector.tensor_add(
                            out=Vs[:, tt],
                            in0=ps[:, 0:260].rearrange("p (h d) -> p h d", h=4),
                            in1=bvS[:],
                        )
                    units.append(v_unit)
                return units

            def sexp_head(qc, h, fill=None):
                ct, pb = h // 2, (h % 2) * 64
                # diag tiles first (kts 4qc..4qc+3), then full tiles 0..4qc-1
                order = list(range(4 * qc, 4 * qc + 4)) + list(range(4 * qc))
                pts = {}
                for i, kt in enumerate(order):
                    oi = kt - 4 * qc
                    q0 = max(oi, 0) * P  # causal trim within chunk
                    sl = i % 2
                    if sl == 0:
                        if fill is not None and i % 6 == 2:
                            fill(1)
                        spair = psS.tile([P, 2, 512], f32, tag="spair")
                        ppair = ptp.tile([P, 2, 512], bf16, tag="ppair")
                    nc.tensor.matmul(
                        spair[:, sl, q0:],
                        KT[pb : pb + 64, ct, kt * P : (kt + 1) * P],
                        QT[pb : pb + 64, ct, qc * 512 + q0 : (qc + 1) * 512],
                        start=True,
                        stop=True,
                        skip_group_check=True,
                    )
                    # exp: diag tiles (first 4 in order) one at a time with
                    # trimmed APs; full tiles batched per aligned pair.
                    if oi >= 0:
                        nc.scalar.activation(
                            ppair[:, sl, q0:], spair[:, sl, q0:], Exp,
                            scale=0.125,
                        )
                        nc.vector.tensor_mul(
                            out=ppair[:, sl, q0 : q0 + P],
                            in0=ppair[:, sl, q0 : q0 + P],
                            in1=maskS[:],
                        )
                    elif sl == 1:
                        nc.scalar.activation(
                            ppair[:].rearrange("p j q -> p (j q)"),
                            spair[:].rearrange("p j q -> p (j q)"),
                            Exp,
                            scale=0.125,
                        )
                    pts[kt] = (ppair, sl)
                return pts

            def av_head(qc, h, pts, ysp):
                ct, pb = h // 2, (h % 2) * 64
                yps = psy.tile([P, 4, 65], f32, tag="yps")
                # qb-outer: each PSUM accumulation group in the shared 2KB
                # zero region fully closes before the next starts (a start
                # marks the whole region pending-zero, so interleaved open
                # groups would wipe each other).
                for qb in range(4):
                    for kt in range(4 * qc + qb + 1):
                        pp, sl = pts[kt]
                        nc.tensor.matmul(
                            yps[:, qb],
                            pp[:, sl, qb * P : (qb + 1) * P],
                            Vs[:, kt, h],
                            start=(kt == 0),
                            stop=(kt == 4 * qc + qb),
                            skip_group_check=True,
                        )
                # Normalize during PSUM evacuation: denominators sit in
                # column 64 (ones-column of V).
                den = rcp.tile([P, 4], f32, tag="den")
                nc.vector.tensor_copy(out=den[:], in_=yps[:, :, 64])
                rec = rcp.tile([P, 4], f32, tag="rec")
                nc.vector.reciprocal_approx_fast(out=rec[:], in_=den[:])
                if debug:
                    nc.sync.dma_start(denD[qc, :, h, 0:2], den[:, 0:2])
                    nc.sync.dma_start(denD[qc, :, h, 2:4], rec[:, 0:2])
                for qb in range(4):
                    nc.vector.tensor_mul(
                        out=ysp[:, ct, qb, pb : pb + 64],
                        in0=yps[:, qb, 0:64],
                        in1=rec[:, qb : qb + 1].to_broadcast((P, 64)),
                    )

            def transpose_ct(qc, ctp, ysp, tp8):
                for qb in range(4):
                    nc.tensor.transpose(
                        tp8[:, ctp * 4 + qb], ysp[:, ctp, qb], identS[:]
                    )
                    nc.vector.tensor_copy(
                        out=ysb[:, ctp, qc * 512 + qb * P : qc * 512 + (qb + 1) * P],
                        in_=tp8[:, ctp * 4 + qb],
                    )

            def outproj_units(qc, act_evac=False):
                units = []
                for ft in range(8):
                    def o_unit(ft=ft):
                        ps = pgen.tile([P, 512], f32, tag="gen")
                        for ct in range(NCT):
                            nc.tensor.matmul(
                                ps[:],
                                woT[:, ct, ft * P : (ft + 1) * P],
                                ysb[:, ct, qc * 512 : (qc + 1) * 512],
                                start=(ct == 0),
                                stop=(ct == NCT - 1),
                            )
                        ob = ostg.tile([P, 512], bf16)
                        if act_evac and ft % 2 == 1:
                            nc.scalar.activation(
                                ob[:], ps[:],
                                mybir.ActivationFunctionType.Copy, scale=1.0,
                            )
                        else:
                            nc.vector.tensor_copy(out=ob[:], in_=ps[:])
                        # partials go to DRAM scratch for the ReduceScatter;
                        # NEVER on gpsimd (collectives own that queue).
                        eng = nc.sync if ft % 2 == 0 else nc.scalar
                        eng.dma_start(
                            ypD[qc][ft * P : (ft + 1) * P, :],
                            ob[:],
                        )
                    units.append(o_unit)
                return units

            # Software-pipelined drive: proj(qc+1) and deferred outproj
            # units are woven into attention(qc) as PE filler at the
            # exp-wait stall points (attention alone is ACT-bound: exp
            # costs 2x the S matmul cycles per column).  proj units must
            # fully drain before the attention chunk that consumes them;
            # outproj units can run any time after their chunk and are
            # held back for the late (filler-starved) chunks.
            for u in proj_units(0):
                u()
            proj_fill = []
            out_fill = []

            def fill(n):
                for _ in range(n):
                    if proj_fill:
                        proj_fill.pop(0)()
                    elif out_fill:
                        out_fill.pop(0)()

            for qc in range(4):
                for u in proj_fill:
                    u()
                proj_fill = proj_units(qc + 1) if qc < 3 else []
                ysp = yspp.tile([P, NCT, 4, P], bf16, tag="ysp")
                tp8 = pst.tile([P, 8, P], bf16, tag="tp8")
                pts0 = sexp_head(qc, 0, fill)
                pts1 = sexp_head(qc, 1, fill)
                fill(1)
                av_head(qc, 0, pts0, ysp)
                fill(1)
                av_head(qc, 1, pts1, ysp)
                pts2 = sexp_head(qc, 2, fill)
                fill(1)
                transpose_ct(qc, 0, ysp, tp8)
                pts3 = sexp_head(qc, 3, fill)
                fill(1)
                av_head(qc, 2, pts2, ysp)
                fill(1)
                av_head(qc, 3, pts3, ysp)
                if debug:
                    nc.sync.dma_start(yspD[qc], ysp[:])
                fill(2)
                transpose_ct(qc, 1, ysp, tp8)
                out_fill += outproj_units(qc, act_evac=(qc == 3))
            for u in proj_fill + out_fill:
                u()
            # ---- ReduceScatter the out-proj partials, chunk by chunk.
            # Tile deps stall each RS until its ypD chunk is fully written;
            # gpsimd has nothing else queued behind these.
            for qc in range(4):
                nc.gpsimd.collective_compute(
                    "ReduceScatter", mybir.AluOpType.add, replica_groups=GROUPS,
                    ins=[ypD[qc][:].opt()], outs=[ysD[qc][:].opt()],
                )
                nc.sync.dma_start(outT[:, qc * 512 : (qc + 1) * 512], ysD[qc][:])
            if debug:
                nc.sync.dma_start(QTD[:], QT[:])
                nc.sync.dma_start(KTD[:], KT[:])
                nc.sync.dma_start(VsD[:], Vs[:])
                nc.sync.dma_start(ysbD[:], ysb[:])
    nc.compile()
    return nc


def make_in_maps(x, Wq, bq, Wk, bk, Wv, bv, Wo, bo):
    ropec = _rope_ct().astype(BF).ravel()
    maskc = _mask_tri().astype(BF).ravel()
    identc = np.eye(P, dtype=np.float32).astype(BF).ravel()
    cflat = np.concatenate([ropec, maskc, identc])  # (SZ_ROPE + 2*SZ_MASK,)
    xTb = [np.asarray(x[b]).T.astype(BF) for b in range(B)]  # [E, T]
    in_maps = []
    for b, g in CORES:
        cs = g * CL
        bvbA = np.empty((P, 4, 65), np.float32)
        bvbA[:, :, 0:64] = bv[cs : cs + CL].reshape(4, 64)[None]
        bvbA[:, :, 64] = 1.0
        # Wv^T padded to 65-wide per-head blocks; col 64 stays 0 so the
        # bias add (1.0 there) plants the ones column of V.
        wvp = np.zeros((E, 260), np.float32)
        wvp[:, :].reshape(E, 4, 65)[:, :, 0:64] = (
            np.asarray(Wv[cs : cs + CL]).T.reshape(E, 4, 64)
        )
        blob = np.empty(SZ_BLOB, BF)
        # rank-g quarters of the batch-shared regions
        blob[OFF_R0:OFF_R1] = xTb[b][g * 256 : (g + 1) * 256, 0:512].ravel()
        blob[OFF_R1:OFF_R2] = xTb[b][g * 256 : (g + 1) * 256, 512:2048].ravel()
        blob[OFF_R2:OFF_W] = cflat[g * SH_R2 : (g + 1) * SH_R2]
        # per-core weights/biases
        blob[OFF_WQ : OFF_WQ + SZ_WQ] = np.asarray(Wq[cs : cs + CL]).T.astype(BF).ravel()
        blob[OFF_WK : OFF_WK + SZ_WQ] = np.asarray(Wk[cs : cs + CL]).T.astype(BF).ravel()
        blob[OFF_WV : OFF_WV + SZ_WV] = wvp.astype(BF).ravel()
        blob[OFF_WO : OFF_WO + SZ_WO] = (
            np.asarray(Wo[:, cs : cs + CL]).T.astype(BF).ravel()
        )
        blob[OFF_BQ : OFF_BQ + SZ_BQ] = (
            np.asarray(bq[cs : cs + CL]).reshape(NCT, P).T.astype(BF).ravel()
        )
        blob[OFF_BK : OFF_BK + SZ_BQ] = (
            np.asarray(bk[cs : cs + CL]).reshape(NCT, P).T.astype(BF).ravel()
        )
        blob[OFF_BV : OFF_BV + SZ_BV] = bvbA.astype(BF).ravel()
        in_maps.append({"blob": blob})
    return in_maps


def assemble_output(results, bo):
    out = np.empty((B, T, E), np.float32)
    for c, (b, g) in enumerate(CORES):
        out[b][:, g * CL : (g + 1) * CL] = np.asarray(
            results[c]["outT"], dtype=np.float32
        ).T
    out += np.asarray(bo, dtype=np.float32)[None, None, :]
    return out


def kernel(x, Wq, bq, Wk, bk, Wv, bv, Wo, bo, _trace=False, _trace_kwargs=None):
    from concourse.bass_utils import run_bass_kernel_spmd

    nc = build_nc()
    in_maps = make_in_maps(x, Wq, bq, Wk, bk, Wv, bv, Wo, bo)
    res = run_bass_kernel_spmd(
        nc, in_maps, list(range(N_CORES)), trace=_trace, **(_trace_kwargs or {})
    )
    out = assemble_output(res.results, bo)
    if _trace:
        return out, res
    return out


# revision 6
# speedup vs baseline: 2.4548x; 2.4548x over previous
"""Self-contained Trainium2 Bass kernel for nn_CausalSelfAttention_59528246722829 (v3).

Sharding: 8 cores = 2 batches x 4 head-groups (4 heads / 256 channels each).
Each core computes Q/K/V projections for its head group, causal attention,
and a partial output projection.  The host sums the 4 partials per batch and
adds the output bias (tensor-parallel partial-sum unshard).

v3 changes vs v2 (dispatch-overhead driven):
  - Measured per-dispatch cost is dominated by argument binding (~30 us per
    arg per exec) plus ~11-16 us/MiB of arg bytes; device compute is almost
    fully hidden.  All 12 inputs are therefore packed into ONE flat bf16
    blob (6.6 MiB) -> 2 args total (blob in, outT out) instead of 13.
  - Biases ship as bf16 inside the blob and are up-converted to fp32 on
    device with three tiny tensor_copy ops (ones-column of V is exactly
    representable; bias magnitudes ~0.01 so bf16 rounding is negligible).

v2 design (cost-model driven, unchanged here):
  - S^T matmuls trimmed to the causal triangle, exp trimmed the same way,
    causal mask shrinks to a single [128,128] lower-tri block.
  - AV computed in natural [query, d] layout with a 65-wide V tile whose
    ones-column yields the softmax denominator; reciprocal + broadcast
    multiply normalizes during PSUM evacuation.
  - y^T recovered with PE transpose instructions; qc-outer / head-inner
    loop order so out-projection overlaps the next chunk's attention.
  - proj/outproj units woven into attention as PE filler at exp-wait
    stall points (attention alone is ACT-bound).

Device compute dtype: bf16 matmul operands, fp32 PSUM accumulation.
Layouts (partition x free):
  xT   [128, 8, 2048]  x^T    (e on partition)        bf16
  wT   [128, 8, 256]   Wq/Wk^T (e part, c free)       bf16
  wvP  [128, 8, 260]   Wv^T padded per-head to 65     bf16 (host-padded)
  woT  [128, 2, 1024]  Wo^T   (c part, f free)        bf16
  QT/KT[128, 2, 2048]  Q^T/K^T (c part, t free)       bf16
  Vs   [128, 16, 4, 65] V natural (t part) + ones col bf16
  ysb  [128, 2, 2048]  y^T    (c part, t free)        bf16
"""

import numpy as np
import ml_dtypes

BF = ml_dtypes.bfloat16
B, T, E, H, DH = 2, 2048, 1024, 16, 64
P, NE, CL, NCT = 128, 8, 256, 2
ROPE_BASE = 10000.0
N_CORES = 8
CORES = [(b, g) for b in range(B) for g in range(4)]  # (batch, head-group)

# ---- blob layout (elements, bf16) ----
SZ_X, SZ_WQ, SZ_WV, SZ_WO = E * T, E * CL, E * 260, CL * E
SZ_BQ, SZ_BV = P * NCT, P * 4 * 65
SZ_ROPE, SZ_MASK = P * T, P * P
OFF_X = 0
OFF_WQ = OFF_X + SZ_X
OFF_WK = OFF_WQ + SZ_WQ
OFF_WV = OFF_WK + SZ_WQ
OFF_WO = OFF_WV + SZ_WV
OFF_BQ = OFF_WO + SZ_WO
OFF_BK = OFF_BQ + SZ_BQ
OFF_BV = OFF_BK + SZ_BQ
OFF_ROPE = OFF_BV + SZ_BV
OFF_MASK = OFF_ROPE + SZ_ROPE
OFF_IDENT = OFF_MASK + SZ_MASK
SZ_BLOB = OFF_IDENT + SZ_MASK


def _rope_ct():
    """C^T[p, t] = cos(theta) + sin(theta), theta = t * base^(-2*(p%32)/64).

    The reference's buggy rope (d_param = n_embd slices the heads axis)
    degenerates to an elementwise multiply of Q and K by this factor.
    """
    i = (np.arange(P) % 32).astype(np.float64)
    t = np.arange(T, dtype=np.float64)
    th = t[None, :] * (ROPE_BASE ** (-2.0 * i[:, None] / DH))
    return (np.cos(th) + np.sin(th)).astype(np.float32)


def _mask_tri():
    """mask[p, q] = 1 if q >= p else 0 -- the 128x128 diagonal block mask."""
    p = np.arange(P)[:, None]
    q = np.arange(P)[None, :]
    return (q >= p).astype(np.float32)


def build_nc(debug=False):
    import concourse.bass as bass
    import concourse.tile as tile
    from concourse import mybir, bacc
    from contextlib import ExitStack

    f32, bf16 = mybir.dt.float32, mybir.dt.bfloat16
    Exp = mybir.ActivationFunctionType.Exp

    nc = bacc.Bacc("TRN2")
    if debug:
        QTD = nc.declare_dram_parameter("QTD", [P, NCT, T], bf16, isOutput=True)
        KTD = nc.declare_dram_parameter("KTD", [P, NCT, T], bf16, isOutput=True)
        VsD = nc.declare_dram_parameter("VsD", [P, 16, 4, 65], bf16, isOutput=True)
        ysbD = nc.declare_dram_parameter("ysbD", [P, NCT, T], bf16, isOutput=True)
        denD = nc.declare_dram_parameter("denD", [4, P, 4, 4], f32, isOutput=True)
        yspD = nc.declare_dram_parameter("yspD", [4, P, NCT, 4, P], bf16, isOutput=True)
    # Single packed input: every operand pre-transposed + pre-cast to bf16.
    blob = nc.declare_dram_parameter("blob", [SZ_BLOB], bf16, isOutput=False)
    outT = nc.declare_dram_parameter("outT", [E, T], bf16, isOutput=True)

    def bsl(off, sz):
        return blob[off : off + sz]

    with ExitStack() as ctx:
        tc = ctx.enter_context(tile.TileContext(nc))
        cst = ctx.enter_context(tc.tile_pool(name="cst", bufs=1))

        # ---- Phase 0: stream inputs across the three DGE queues ----
        # SP: wq, x chunks.  ACT: wk, x chunks (frees early for exp).
        # Pool: constants + wv + wo.
        xT = cst.tile([P, NE, T], bf16)
        xsrc = bsl(OFF_X, SZ_X).rearrange("(et p t) -> p et t", p=P, t=T)
        wT = {}
        wT["q"] = cst.tile([P, NE, CL], bf16, name="wTq", tag="wTq")
        nc.sync.dma_start(
            wT["q"][:], bsl(OFF_WQ, SZ_WQ).rearrange("(et p c) -> p et c", p=P, c=CL)
        )
        wT["k"] = cst.tile([P, NE, CL], bf16, name="wTk", tag="wTk")
        nc.scalar.dma_start(
            wT["k"][:], bsl(OFF_WK, SZ_WQ).rearrange("(et p c) -> p et c", p=P, c=CL)
        )
        # x tq0 split across both queues so the first projection starts ASAP
        nc.sync.dma_start(xT[:, 0:4, 0:512], xsrc[:, 0:4, 0:512])
        nc.scalar.dma_start(xT[:, 4:8, 0:512], xsrc[:, 4:8, 0:512])
        nc.sync.dma_start(xT[:, :, 512:1024], xsrc[:, :, 512:1024])
        nc.scalar.dma_start(xT[:, :, 1024:1536], xsrc[:, :, 1024:1536])
        nc.sync.dma_start(xT[:, :, 1536:2048], xsrc[:, :, 1536:2048])
        ropeS = cst.tile([P, T], bf16)
        nc.gpsimd.dma_start(
            ropeS[:], bsl(OFF_ROPE, SZ_ROPE).rearrange("(p t) -> p t", p=P)
        )
        bqSb = cst.tile([P, NCT], bf16)
        nc.gpsimd.dma_start(
            bqSb[:], bsl(OFF_BQ, SZ_BQ).rearrange("(p n) -> p n", p=P)
        )
        bkSb = cst.tile([P, NCT], bf16)
        nc.gpsimd.dma_start(
            bkSb[:], bsl(OFF_BK, SZ_BQ).rearrange("(p n) -> p n", p=P)
        )
        bvSb = cst.tile([P, 4, 65], bf16)
        nc.gpsimd.dma_start(
            bvSb[:], bsl(OFF_BV, SZ_BV).rearrange("(p h d) -> p h d", p=P, h=4)
        )
        maskS = cst.tile([P, P], bf16)
        nc.gpsimd.dma_start(
            maskS[:], bsl(OFF_MASK, SZ_MASK).rearrange("(p q) -> p q", p=P)
        )
        identS = cst.tile([P, P], bf16)
        nc.gpsimd.dma_start(
            identS[:], bsl(OFF_IDENT, SZ_MASK).rearrange("(p q) -> p q", p=P)
        )
        wvP = cst.tile([P, NE, 4, 65], bf16)
        nc.gpsimd.dma_start(
            wvP[:].rearrange("p et h d -> p et (h d)"),
            bsl(OFF_WV, SZ_WV).rearrange("(et p c) -> p et c", p=P, c=260),
        )
        woT = cst.tile([P, NCT, E], bf16)
        nc.gpsimd.dma_start(
            woT[:], bsl(OFF_WO, SZ_WO).rearrange("(ct p f) -> p ct f", p=P, f=E)
        )
        # up-convert biases to fp32 once (broadcast adds below want fp32)
        bqS = cst.tile([P, NCT], f32)
        nc.vector.tensor_copy(out=bqS[:], in_=bqSb[:])
        bkS = cst.tile([P, NCT], f32)
        nc.vector.tensor_copy(out=bkS[:], in_=bkSb[:])
        bvS = cst.tile([P, 4, 65], f32)
        nc.vector.tensor_copy(out=bvS[:], in_=bvSb[:])

        QT = cst.tile([P, NCT, T], bf16)
        KT = cst.tile([P, NCT, T], bf16)
        Vs = cst.tile([P, 16, 4, 65], bf16)
        ysb = cst.tile([P, NCT, T], bf16)

        # ---- Fused phases, t-chunk major.  PE stream per chunk qc:
        #   [proj(0) first] Sexp(h0) Sexp(h1) AV(h0) AV(h1) Sexp(h2) Sexp(h3)
        #   T(ct0) AV(h2) AV(h3) | proj(qc+1) | T(ct1) outproj(qc)
        # Head-pairing hides each head's exp+mask chain under the next
        # head's S matmuls; T(ct1)+outproj(qc) hide the last evac chain
        # under proj(qc+1).  Within a head, diagonal (trimmed) S tiles run
        # FIRST so their per-tile exps overlap the full-tile S stream.
        # PSUM banks: big(proj/S-pairs/outproj) 2x2 + psy 2 + pst 1 = 7.
        with (
            tc.tile_pool(name="pgen", bufs=2, space="PSUM") as pgen,
            tc.tile_pool(name="psS", bufs=2, space="PSUM") as psS,
            tc.tile_pool(name="psy", bufs=1, space="PSUM") as psy,
            tc.tile_pool(name="pst", bufs=1, space="PSUM") as pst,
            tc.tile_pool(name="stg1", bufs=4) as stg1,
            tc.tile_pool(name="ptp", bufs=25) as ptp,
            tc.tile_pool(name="ysp", bufs=2) as yspp,
            tc.tile_pool(name="rcp", bufs=4) as rcp,
            tc.tile_pool(name="ostg", bufs=4) as ostg,
        ):
            def proj_units(tq):
                units = []
                for nm, dst, bS in (("q", QT, bqS), ("k", KT, bkS)):
                    for ct in range(NCT):
                        def qk_unit(nm=nm, dst=dst, bS=bS, ct=ct):
                            ps = pgen.tile([P, 512], f32, tag="gen")
                            for et in range(NE):
                                nc.tensor.matmul(
                                    ps[:],
                                    wT[nm][:, et, ct * P : (ct + 1) * P],
                                    xT[:, et, tq * 512 : (tq + 1) * 512],
                                    start=(et == 0),
                                    stop=(et == NE - 1),
                                )
                            tmp = stg1.tile([P, 512], bf16)
                            nc.vector.tensor_add(
                                out=tmp[:],
                                in0=ps[:],
                                in1=bS[:, ct : ct + 1].to_broadcast((P, 512)),
                            )
                            nc.vector.tensor_mul(
                                out=dst[:, ct, tq * 512 : (tq + 1) * 512],
                                in0=tmp[:],
                                in1=ropeS[:, tq * 512 : (tq + 1) * 512],
                            )
                        units.append(qk_unit)
                for tt in range(4 * tq, 4 * tq + 4):
                    def v_unit(tt=tt):
                        ps = pgen.tile([P, 512], f32, tag="gen")
                        for et in range(NE):
                            nc.tensor.matmul(
                                ps[:, 0:260],
                                xT[:, et, tt * P : (tt + 1) * P],
                                wvP[:, et].rearrange("p h d -> p (h d)"),
                                start=(et == 0),
                                stop=(et == NE - 1),
                            )
                        nc.vector.tensor_add(
                            out=Vs[:, tt],
                            in0=ps[:, 0:260].rearrange("p (h d) -> p h d", h=4),
                            in1=bvS[:],
                        )
                    units.append(v_unit)
                return units

            def sexp_head(qc, h, fill=None):
                ct, pb = h // 2, (h % 2) * 64
                # diag tiles first (kts 4qc..4qc+3), then full tiles 0..4qc-1
                order = list(range(4 * qc, 4 * qc + 4)) + list(range(4 * qc))
                pts = {}
                for i, kt in enumerate(order):
                    oi = kt - 4 * qc
                    q0 = max(oi, 0) * P  # causal trim within chunk
                    sl = i % 2
                    if sl == 0:
                        if fill is not None and i % 6 == 2:
                            fill(1)
                        spair = psS.tile([P, 2, 512], f32, tag="spair")
                        ppair = ptp.tile([P, 2, 512], bf16, tag="ppair")
                    nc.tensor.matmul(
                        spair[:, sl, q0:],
                        KT[pb : pb + 64, ct, kt * P : (kt + 1) * P],
                        QT[pb : pb + 64, ct, qc * 512 + q0 : (qc + 1) * 512],
                        start=True,
                        stop=True,
                        skip_group_check=True,
                    )
                    # exp: diag tiles (first 4 in order) one at a time with
                    # trimmed APs; full tiles batched per aligned pair.
                    if oi >= 0:
                        nc.scalar.activation(
                            ppair[:, sl, q0:], spair[:, sl, q0:], Exp,
                            scale=0.125,
                        )
                        nc.vector.tensor_mul(
                            out=ppair[:, sl, q0 : q0 + P],
                            in0=ppair[:, sl, q0 : q0 + P],
                            in1=maskS[:],
                        )
                    elif sl == 1:
                        nc.scalar.activation(
                            ppair[:].rearrange("p j q -> p (j q)"),
                            spair[:].rearrange("p j q -> p (j q)"),
                            Exp,
                            scale=0.125,
                        )
                    pts[kt] = (ppair, sl)
                return pts

            def av_head(qc, h, pts, ysp):
                ct, pb = h // 2, (h % 2) * 64
                yps = psy.tile([P, 4, 65], f32, tag="yps")
                # qb-outer: each PSUM accumulation group in the shared 2KB
                # zero region fully closes before the next starts (a start
                # marks the whole region pending-zero, so interleaved open
                # groups would wipe each other).
                for qb in range(4):
                    for kt in range(4 * qc + qb + 1):
                        pp, sl = pts[kt]
                        nc.tensor.matmul(
                            yps[:, qb],
                            pp[:, sl, qb * P : (qb + 1) * P],
                            Vs[:, kt, h],
                            start=(kt == 0),
                            stop=(kt == 4 * qc + qb),
                            skip_group_check=True,
                        )
                # Normalize during PSUM evacuation: denominators sit in
                # column 64 (ones-column of V).
                den = rcp.tile([P, 4], f32, tag="den")
                nc.vector.tensor_copy(out=den[:], in_=yps[:, :, 64])
                rec = rcp.tile([P, 4], f32, tag="rec")
                nc.vector.reciprocal_approx_fast(out=rec[:], in_=den[:])
                if debug:
                    nc.sync.dma_start(denD[qc, :, h, 0:2], den[:, 0:2])
                    nc.sync.dma_start(denD[qc, :, h, 2:4], rec[:, 0:2])
                for qb in range(4):
                    nc.vector.tensor_mul(
                        out=ysp[:, ct, qb, pb : pb + 64],
                        in0=yps[:, qb, 0:64],
                        in1=rec[:, qb : qb + 1].to_broadcast((P, 64)),
                    )

            def transpose_ct(qc, ctp, ysp, tp8):
                for qb in range(4):
                    nc.tensor.transpose(
                        tp8[:, ctp * 4 + qb], ysp[:, ctp, qb], identS[:]
                    )
                    nc.vector.tensor_copy(
                        out=ysb[:, ctp, qc * 512 + qb * P : qc * 512 + (qb + 1) * P],
                        in_=tp8[:, ctp * 4 + qb],
                    )

            def outproj_units(qc, act_evac=False):
                units = []
                for ft in range(8):
                    def o_unit(ft=ft):
                        ps = pgen.tile([P, 512], f32, tag="gen")
                        for ct in range(NCT):
                            nc.tensor.matmul(
                                ps[:],
                                woT[:, ct, ft * P : (ft + 1) * P],
                                ysb[:, ct, qc * 512 : (qc + 1) * 512],
                                start=(ct == 0),
                                stop=(ct == NCT - 1),
                            )
                        ob = ostg.tile([P, 512], bf16)
                        if act_evac and ft % 2 == 1:
                            nc.scalar.activation(
                                ob[:], ps[:],
                                mybir.ActivationFunctionType.Copy, scale=1.0,
                            )
                        else:
                            nc.vector.tensor_copy(out=ob[:], in_=ps[:])
                        eng = nc.sync if ft % 2 == 0 else nc.gpsimd
                        eng.dma_start(
                            outT[ft * P : (ft + 1) * P, qc * 512 : (qc + 1) * 512],
                            ob[:],
                        )
                    units.append(o_unit)
                return units

            # Software-pipelined drive: proj(qc+1) and deferred outproj
            # units are woven into attention(qc) as PE filler at the
            # exp-wait stall points (attention alone is ACT-bound: exp
            # costs 2x the S matmul cycles per column).  proj units must
            # fully drain before the attention chunk that consumes them;
            # outproj units can run any time after their chunk and are
            # held back for the late (filler-starved) chunks.
            for u in proj_units(0):
                u()
            proj_fill = []
            out_fill = []

            def fill(n):
                for _ in range(n):
                    if proj_fill:
                        proj_fill.pop(0)()
                    elif out_fill:
                        out_fill.pop(0)()

            for qc in range(4):
                for u in proj_fill:
                    u()
                proj_fill = proj_units(qc + 1) if qc < 3 else []
                ysp = yspp.tile([P, NCT, 4, P], bf16, tag="ysp")
                tp8 = pst.tile([P, 8, P], bf16, tag="tp8")
                pts0 = sexp_head(qc, 0, fill)
                pts1 = sexp_head(qc, 1, fill)
                fill(1)
                av_head(qc, 0, pts0, ysp)
                fill(1)
                av_head(qc, 1, pts1, ysp)
                pts2 = sexp_head(qc, 2, fill)
                fill(1)
                transpose_ct(qc, 0, ysp, tp8)
                pts3 = sexp_head(qc, 3, fill)
                fill(1)
                av_head(qc, 2, pts2, ysp)
                fill(1)
                av_head(qc, 3, pts3, ysp)
                if debug:
                    nc.sync.dma_start(yspD[qc], ysp[:])
                fill(2)
                transpose_ct(qc, 1, ysp, tp8)
                out_fill += outproj_units(qc, act_evac=(qc == 3))
            for u in proj_fill + out_fill:
                u()
            if debug:
                nc.sync.dma_start(QTD[:], QT[:])
                nc.sync.dma_start(KTD[:], KT[:])
                nc.sync.dma_start(VsD[:], Vs[:])
                nc.sync.dma_start(ysbD[:], ysb[:])
    nc.compile()
    return nc


def make_in_maps(x, Wq, bq, Wk, bk, Wv, bv, Wo, bo):
    ropec = _rope_ct().astype(BF).ravel()
    maskc = _mask_tri().astype(BF).ravel()
    identc = np.eye(P, dtype=np.float32).astype(BF).ravel()
    xTb = [np.asarray(x[b]).T.astype(BF).ravel() for b in range(B)]
    in_maps = []
    for b, g in CORES:
        cs = g * CL
        bvbA = np.empty((P, 4, 65), np.float32)
        bvbA[:, :, 0:64] = bv[cs : cs + CL].reshape(4, 64)[None]
        bvbA[:, :, 64] = 1.0
        # Wv^T padded to 65-wide per-head blocks; col 64 stays 0 so the
        # bias add (1.0 there) plants the ones column of V.
        wvp = np.zeros((E, 260), np.float32)
        wvp[:, :].reshape(E, 4, 65)[:, :, 0:64] = (
            np.asarray(Wv[cs : cs + CL]).T.reshape(E, 4, 64)
        )
        blob = np.empty(SZ_BLOB, BF)
        blob[OFF_X : OFF_X + SZ_X] = xTb[b]
        blob[OFF_WQ : OFF_WQ + SZ_WQ] = np.asarray(Wq[cs : cs + CL]).T.astype(BF).ravel()
        blob[OFF_WK : OFF_WK + SZ_WQ] = np.asarray(Wk[cs : cs + CL]).T.astype(BF).ravel()
        blob[OFF_WV : OFF_WV + SZ_WV] = wvp.astype(BF).ravel()
        blob[OFF_WO : OFF_WO + SZ_WO] = (
            np.asarray(Wo[:, cs : cs + CL]).T.astype(BF).ravel()
        )
        blob[OFF_BQ : OFF_BQ + SZ_BQ] = (
            np.asarray(bq[cs : cs + CL]).reshape(NCT, P).T.astype(BF).ravel()
        )
        blob[OFF_BK : OFF_BK + SZ_BQ] = (
            np.asarray(bk[cs : cs + CL]).reshape(NCT, P).T.astype(BF).ravel()
        )
        blob[OFF_BV : OFF_BV + SZ_BV] = bvbA.astype(BF).ravel()
        blob[OFF_ROPE : OFF_ROPE + SZ_ROPE] = ropec
        blob[OFF_MASK : OFF_MASK + SZ_MASK] = maskc
        blob[OFF_IDENT : OFF_IDENT + SZ_MASK] = identc
        in_maps.append({"blob": blob})
    return in_maps


def assemble_output(results, bo):
    out = np.zeros((B, T, E), np.float32)
    for c, (b, g) in enumerate(CORES):
        out[b] += np.asarray(results[c]["outT"], dtype=np.float32).T
    out += np.asarray(bo, dtype=np.float32)[None, None, :]
    return out


def kernel(x, Wq, bq, Wk, bk, Wv, bv, Wo, bo, _trace=False, _trace_kwargs=None):
    from concourse.bass_utils import run_bass_kernel_spmd

    nc = build_nc()
    in_maps = make_in_maps(x, Wq, bq, Wk, bk, Wv, bv, Wo, bo)
    res = run_bass_kernel_spmd(
        nc, in_maps, list(range(N_CORES)), trace=_trace, **(_trace_kwargs or {})
    )
    out = assemble_output(res.results, bo)
    if _trace:
        return out, res
    return out
